# revision 1
# baseline (speedup 1.0000x reference)
"""Causal self-attention (B=4, T=2048, C=1024, H=16) on 8 TRN2 NeuronCores.

Sharding: core c handles batch b=c//2 and head-half hh=c%2 (8 heads).
Each core computes q/k/v projections for its heads, causal attention, and a
partial output projection (row-parallel w_proj); the host sums the two
partials per batch.

v3 design (all-fp16 matmul datapath, fully phase-overlapped):
- x streamed from DRAM once per 512-query chunk in fp16, feeding q/k/v
  projections; a single j-loop interleaves projections, attention and the
  (one-chunk-delayed) output projection so the PE never drains.
- qk psum is evicted by ACT (Identity + per-partition bias) so the PSUM slot
  recycles in ~600ns and RoPE runs on DVE in 2-byte fast mode; rotate-half
  uses two strided partition-swap DMAs on the gpsimd queue.
- attn@v uses a ones-column in v for softmax denominators; v bias and output
  bias fold to the host (softmax rows sum to 1).
- exp on ACT writes fp16 probs; mask via DVE fp16 multiply; attnV trails
  scores by SKEW key tiles to hide exp latency.
"""

import sys

sys.path.insert(0, "/opt/trn_rl_repo")

from contextlib import ExitStack

import numpy as np

import concourse.bass as bass
import concourse.tile as tile
from concourse import bacc, mybir
from concourse.bass_utils import run_bass_kernel_spmd

F32 = mybir.dt.float32
F16 = mybir.dt.float16
AL = mybir.AluOpType
AF = mybir.ActivationFunctionType

B, T, C, H, HD = 4, 2048, 1024, 16, 64
NCORE = 8
HH = H // 2  # heads per core
NP = HH // 2  # head pairs per core
KC = C // 128  # contraction chunks
NT = T // 128  # 128-row time tiles
NQC = T // 512  # 512-query chunks
ROPE_THETA = 10000.0
SKEW = 5  # attnV trails scores by this many key tiles

_CACHE = {}


def _build_module():
    nc = bacc.Bacc("TRN2", target_bir_lowering=False, debug=False)

    xT = nc.dram_tensor("xT", [C, T], F16, kind="ExternalInput")
    wq = nc.dram_tensor("wq", [C, 512], F16, kind="ExternalInput")
    wk = nc.dram_tensor("wk", [C, 512], F16, kind="ExternalInput")
    wv = nc.dram_tensor("wv", [C, 512], F16, kind="ExternalInput")
    wp = nc.dram_tensor("wp", [512, C], F16, kind="ExternalInput")
    bqk = nc.dram_tensor("bqk", [2, NP, 128], F32, kind="ExternalInput")
    cosr = nc.dram_tensor("cosr", [128, T], F16, kind="ExternalInput")
    sinp = nc.dram_tensor("sinp", [128, T], F16, kind="ExternalInput")
    mask = nc.dram_tensor("mask", [128, 2, 128], F16, kind="ExternalInput")
    onesc = nc.dram_tensor("onesc", [128, NT, HH, 1], F16, kind="ExternalInput")
    y = nc.dram_tensor("y", [T, C], F16, kind="ExternalOutput")

    with tile.TileContext(nc) as tc, ExitStack() as ctx:
        consts = ctx.enter_context(tc.tile_pool(name="consts", bufs=1))
        persist = ctx.enter_context(tc.tile_pool(name="persist", bufs=1))
        xp = ctx.enter_context(tc.tile_pool(name="xp", bufs=3))
        rp = ctx.enter_context(tc.tile_pool(name="rp", bufs=2))
        ptp = ctx.enter_context(tc.tile_pool(name="ptp", bufs=SKEW + 2))
        nrm = ctx.enter_context(tc.tile_pool(name="nrm", bufs=2))
        yp = ctx.enter_context(tc.tile_pool(name="yp", bufs=2))
        bigp = ctx.enter_context(tc.tile_pool(name="bigp", bufs=3, space="PSUM"))
        op = ctx.enter_context(tc.tile_pool(name="op", bufs=1, space="PSUM"))

        # ---- constants: declared here, loaded in the prologue across the
        # three DMA-capable queues (sync / scalar / gpsimd) ----
        bqk_sb = consts.tile([128, 2, NP], F32)
        wq_sb = consts.tile([128, KC, 512], F16)
        wk_sb = consts.tile([128, KC, 512], F16)
        cos_sb = consts.tile([128, T], F16)
        sin_sb = consts.tile([128, T], F16)
        mask_sb = consts.tile([128, 2, 128], F16)
        wv_sb = consts.tile([128, KC, 512], F16)
        wp_sb = consts.tile([128, 4, C], F16)

        # ---- persistent activations ----
        qT = persist.tile([128, NP, T], F16)
        kT = persist.tile([128, NP, T], F16)
        vp = persist.tile([128, NT, HH, 65], F16)
        OT = persist.tile([128, NP, T], F16)

        def emit_oproj_tt(j, tt):
            if True:
                t0 = j * 512 + tt * 128
                yps = bigp.tile([128, 2, 512], F32, tag="big")
                for kc in range(4):
                    for nn in range(2):
                        nc.tensor.matmul(
                            yps[:, nn, :],
                            OT[:, kc, t0 : t0 + 128],
                            wp_sb[:, kc, nn * 512 : (nn + 1) * 512],
                            start=(kc == 0),
                            stop=(kc == 3),
                        )
                y16 = yp.tile([128, C], F16, tag="y16")
                nc.vector.tensor_copy(y16[:], yps.rearrange("p a n -> p (a n)"))
                nc.sync.dma_start(out=y[t0 : t0 + 128, :], in_=y16[:])

        xcs = {}

        def emit_xc(j, parts):
            xc = xp.tile([128, KC, 512], F16, tag="xc")
            nk = slice(j * 512, (j + 1) * 512)
            src = xT.rearrange("(kc p) t -> p kc t", p=128)[:, :, nk]
            for eng, k0, k1 in parts:
                for kc in range(k0, k1):
                    eng.dma_start(
                        out=xc[:, kc : kc + 1, :], in_=src[:, kc : kc + 1, :]
                    )
            xcs[j] = xc

        def emit_qkproj_pair(j, p):
            nk = slice(j * 512, (j + 1) * 512)
            xc = xcs[j]
            if True:
                qkps = bigp.tile([128, 2, 512], F32, tag="big")
                for which, wsb in ((0, wq_sb), (1, wk_sb)):
                    for kc in range(KC):
                        nc.tensor.matmul(
                            qkps[:, which, :],
                            wsb[:, kc, p * 128 : (p + 1) * 128],
                            xc[:, kc, :],
                            start=(kc == 0),
                            stop=(kc == KC - 1),
                        )
                for which, dstT in ((0, qT), (1, kT)):
                    # evict psum + bias on ACT; frees the psum slot fast and
                    # keeps the DVE ops in 2-byte fast mode
                    bap = bqk_sb[:, which, p : p + 1]
                    qsf = rp.tile([128, 512], F16, tag="qsf")
                    nc.scalar.activation(
                        qsf[:], qkps[:, which, :], AF.Identity, bias=bap
                    )
                    t1 = rp.tile([128, 512], F16, tag="t1")
                    s1 = rp.tile([128, 512], F16, tag="s1")
                    s2 = rp.tile([128, 512], F16, tag="s2")
                    nc.vector.tensor_mul(t1[:], qsf[:], cos_sb[:, nk])
                    nc.vector.tensor_mul(s1[:], qsf[:], sin_sb[:, nk])
                    # rotate-half via 32-row partition-swap DMAs
                    for o0, i0 in ((0, 32), (32, 0), (64, 96), (96, 64)):
                        nc.gpsimd.dma_start(
                            out=s2[o0 : o0 + 32, :], in_=s1[i0 : i0 + 32, :]
                        )
                    nc.vector.tensor_add(dstT[:, p, nk], t1[:], s2[:])

        def emit_vproj(j):
            xc = xcs.pop(j)
            for half in range(2):
                vps = bigp.tile([128, 2, 512], F32, tag="big")
                for sub in range(2):
                    off = (half * 2 + sub) * 128
                    for kc in range(KC):
                        nc.tensor.matmul(
                            vps[:, sub, :],
                            xc[:, kc, off : off + 128],
                            wv_sb[:, kc, :],
                            start=(kc == 0),
                            stop=(kc == KC - 1),
                        )
                kt0 = 4 * j + half * 2
                nc.scalar.copy(
                    vp[:, kt0 : kt0 + 2, :, 0:64],
                    vps.rearrange("p a (h d) -> p a h d", h=HH),
                )

        def emit_attention_pair(j, p):
            oA = op.tile([65, 512], F32, tag="oA")
            oB = op.tile([65, 512], F32, tag="oB")
            nkt = 4 * (j + 1)
            pend = []

            def attn_v(kt, pt, span, co):
                for h, o in ((0, oA), (1, oB)):
                    nc.tensor.matmul(
                        o[:, co:512],
                        vp[:, kt, p * 2 + h, :],
                        pt[:, h, 0:span],
                        start=(kt == 0),
                        stop=(kt == nkt - 1),
                    )

            for kt in range(nkt):
                i = kt - 4 * j
                span = 512 if i < 0 else 512 - 128 * i
                co = 512 - span
                q0 = j * 512 + co
                sc = bigp.tile([128, 2, 512], F32, tag="big")
                for h in range(2):
                    nc.tensor.matmul(
                        sc[:, h, 0:span],
                        kT[h * 64 : (h + 1) * 64, p,
                           kt * 128 : (kt + 1) * 128],
                        qT[h * 64 : (h + 1) * 64, p, q0 : q0 + span],
                        start=True,
                        stop=True,
                        tile_position=(h * 64, 0),
                    )
                pt = ptp.tile([128, 2, 512], F16, tag="pt")
                nc.scalar.activation(
                    pt[:, :, 0:span], sc[:, :, 0:span], AF.Exp
                )
                if i >= 0:
                    nc.vector.tensor_mul(
                        pt[:, :, 0:128], pt[:, :, 0:128], mask_sb[:]
                    )
                pend.append((kt, pt, span, co))
                if len(pend) > SKEW:
                    attn_v(*pend.pop(0))
            while pend:
                attn_v(*pend.pop(0))

            # normalize: divide by the ones-column sums (psum row 64)
            jq = slice(j * 512, (j + 1) * 512)
            for h, o in ((0, oA), (1, oB)):
                dn = nrm.tile([1, 512], F32, tag=f"dn{h}")
                nc.vector.tensor_copy(dn[:], o[64:65, :])
                dd = nrm.tile([64, 8], F32, tag=f"dd{h}")
                nc.sync.dma_start(
                    out=dd[:],
                    in_=dn.rearrange("p (a b) -> p a b", a=64),
                )
                rr = nrm.tile([64, 8], F32, tag=f"rr{h}")
                nc.vector.reciprocal(rr[:], dd[:])
                dr = nrm.tile([1, 512], F32, tag=f"dr{h}")
                nc.sync.dma_start(
                    out=dr.rearrange("p (a b) -> p a b", a=64), in_=rr[:]
                )
                rb = nrm.tile([64, 512], F32, tag=f"rb{h}")
                nc.gpsimd.partition_broadcast(rb[:], dr[:])
                nc.vector.tensor_mul(
                    OT[h * 64 : (h + 1) * 64, p, jq], o[0:64, :], rb[:]
                )

        # prologue: stripe the loads over the three DMA-capable queues so the
        # first matmul can start in ~8us.
        # sync: wq, wp; scalar: xc0-half, bqk, cos, mask, ones; gpsimd: xc0-half, wk, sin, wv
        wqr = wq.rearrange("(kc p) n -> p kc n", p=128)
        for kc in range(0, KC, 2):
            nc.sync.dma_start(
                out=wq_sb[:, kc : kc + 2, :], in_=wqr[:, kc : kc + 2, :]
            )
        emit_xc(0, [(nc.scalar, 0, 4), (nc.gpsimd, 4, 8)])
        nc.scalar.dma_start(out=bqk_sb[:], in_=bqk.rearrange("a p r -> r a p"))
        wkr = wk.rearrange("(kc p) n -> p kc n", p=128)
        for kc in range(0, KC, 4):
            nc.gpsimd.dma_start(
                out=wk_sb[:, kc : kc + 4, :], in_=wkr[:, kc : kc + 4, :]
            )
        nc.scalar.dma_start(out=cos_sb[:], in_=cosr[:])
        nc.gpsimd.dma_start(out=sin_sb[:], in_=sinp[:])
        nc.sync.dma_start(out=wp_sb[:], in_=wp.rearrange("(kc r) n -> r kc n", r=128))
        nc.scalar.dma_start(out=mask_sb[:], in_=mask[:])
        nc.gpsimd.dma_start(out=wv_sb[:], in_=wv.rearrange("(kc p) n -> p kc n", p=128))
        nc.scalar.dma_start(out=vp[:, :, :, 64:65], in_=onesc[:])
        for p in range(NP):
            emit_qkproj_pair(0, p)
        emit_xc(1, [(nc.sync, 0, 8)])
        emit_vproj(0)

        for j in range(NQC):
            if j + 2 < NQC:
                emit_xc(j + 2, [(nc.sync, 0, 8)])
            last = j == NQC - 1
            fillers = []
            if j > 0:
                fillers += [lambda tt=tt: emit_oproj_tt(j - 1, tt)
                            for tt in range(4)]
            if not last:
                fillers += [lambda p=p: emit_qkproj_pair(j + 1, p)
                            for p in range(NP)]
                fillers.append(lambda: emit_vproj(j + 1))
            # one attention pair, then a slice of filler work, repeated
            nf = len(fillers)
            done = 0
            for p in range(NP):
                emit_attention_pair(j, p)
                want = (p + 1) * nf // NP
                while done < want:
                    fillers[done]()
                    done += 1

        for tt in range(4):
            emit_oproj_tt(NQC - 1, tt)

    nc.compile()
    return nc


def _rope_tables():
    freqs = 1.0 / (ROPE_THETA ** (np.arange(0, HD, 2, dtype=np.float32) / HD))
    ang = np.arange(T, dtype=np.float32)[:, None] * freqs[None, :]  # [T, 32]
    cos = np.cos(ang).T  # [32, T]
    sin = np.sin(ang).T
    cos_rep = np.tile(cos, (4, 1))  # [128, T]
    sgn = np.repeat(np.array([1.0, -1.0, 1.0, -1.0], np.float32), 32)
    sin_pm = np.tile(sin, (4, 1)) * sgn[:, None]
    return cos_rep, sin_pm


def _f16(a):
    return np.ascontiguousarray(a).astype(np.float16)


def _prep_inputs(x, w_qkv, b_qkv, w_proj):
    cos_rep, sin_pm = _rope_tables()
    km = np.arange(128)
    mask1 = (km[:, None] <= km[None, :]).astype(np.float32)  # keep k <= q
    mask2 = np.stack([mask1, mask1], axis=1)  # [128, 2, 128]
    in_maps = []
    for c in range(NCORE):
        b, hh = c // 2, c % 2
        s = hh * 512
        m = {
            "xT": _f16(x[b].T),
            "wq": _f16(w_qkv[:, s : s + 512] / 8.0),
            "wk": _f16(w_qkv[:, C + s : C + s + 512]),
            "wv": _f16(w_qkv[:, 2 * C + s : 2 * C + s + 512]),
            "wp": _f16(w_proj[s : s + 512, :]),
            "bqk": np.stack(
                [
                    b_qkv[s : s + 512].reshape(NP, 128) / 8.0,
                    b_qkv[C + s : C + s + 512].reshape(NP, 128),
                ]
            ).astype(np.float32),
            "onesc": np.ones((128, NT, HH, 1), np.float16),
            "cosr": _f16(cos_rep),
            "sinp": _f16(sin_pm),
            "mask": _f16(mask2),
        }
        in_maps.append(m)
    return in_maps


def _run(x, w_qkv, b_qkv, w_proj, b_proj, trace=False):
    if "nc" not in _CACHE:
        _CACHE["nc"] = _build_module()
    nc = _CACHE["nc"]
    x = np.asarray(x, np.float32)
    w_qkv = np.asarray(w_qkv, np.float32)
    b_qkv = np.asarray(b_qkv, np.float32)
    w_proj = np.asarray(w_proj, np.float32)
    b_proj = np.asarray(b_proj, np.float32)
    in_maps = _prep_inputs(x, w_qkv, b_qkv, w_proj)
    res = run_bass_kernel_spmd(nc, in_maps, core_ids=list(range(NCORE)), trace=trace)
    # host-side: sum row-parallel partials and add the folded biases
    # (attn @ (v + bv) = attn @ v + bv since softmax rows sum to 1)
    ybias = (b_qkv[2 * C :] @ w_proj + b_proj).astype(np.float32)
    out = np.empty((B, T, C), np.float32)
    for b in range(B):
        out[b] = (
            res.results[2 * b]["y"].astype(np.float32)
            + res.results[2 * b + 1]["y"].astype(np.float32)
            + ybias
        )
    return out, res


def kernel(x, w_qkv, b_qkv, w_proj, b_proj, n_heads=16):
    out, _ = _run(x, w_qkv, b_qkv, w_proj, b_proj, trace=False)
    return out

